# revision 55
# baseline (speedup 1.0000x reference)
"""Trainium2 Bass kernel for conditional-adjustment conv (CAConv).

Per sample b: h = relu(c[b] @ mlp_w1 + mlp_b1); adj = h @ mlp_w2 + mlp_b2;
w[b] = conv_w + adj.reshape(Co,Ci,3,3); out[b] = conv2d(x[b], w[b], pad=1) + conv_b.

Sharding: data-parallel over batch, 4 samples per core on 8 cores (SPMD).

Per-core device kernel:
  Stage A (weight gen, f32r): the mlp matmul uses host-replicated w1 (4
  copies at partition offsets 0/32/64/96) so ht lands pre-replicated for
  the packed w2 k-groups in one relu — no replication DMAs. The 17th row
  of c/w1 is ones, so row 16 of w2p — host-set to mlp_b2 + conv_w, both
  permuted — rides along and adj is directly the complete per-sample conv
  weight. adj = hT.T @ w2p streamed as 72 [17->4, 512] f32r matmuls; each
  [4,1024] PSUM quarter is cast to bf16 (alternating DVE / scalar-ACT to
  halve the 4-partition cast serialization) and DMA-scattered (sync HWDGE)
  onto the diagonal blocks of the per-pair block-diagonal bf16 weight tile
  wblk[ci + 64*half, t*128 + 64*half + co].
  Stage B (conv, bf16 in / fp32 PSUM): host-padded bf16 x (130x130) for a
  sample pair lives as [ci(2 samples), h, w] across the 128 partitions.
  Each output chunk po[128, 512] (2 samples x 64 co partitions; 4 h-rows x
  128 w free) accumulates 9 shift-tap K=128 matmuls. Bias is added during
  the PSUM->SBUF copy (bf16 out), then DMA to DRAM; host upcasts to fp32.

  A dozen dummy [1,512] matmuls at t~0 keep the PE clock ramping (p-state
  reaches full speed only after 3us of continuous execution) so the first
  real matmuls run at full rate. The scalar act-table is warmed at t~0 so
  the relu isn't gated on a late table load.

  DMA queues: sync HWDGE = consts + weight scatters + tail stores; ACT
  HWDGE (scalar) = w2 chunks + output stores; GPSIMD SWDGE = wblk
  zero-fills + bulk x loads (latency-tolerant, ~1us fixed overhead each).
"""

import sys

if "/opt/trn_rl_repo" not in sys.path:
    sys.path.insert(0, "/opt/trn_rl_repo")

import numpy as np

B = 32
NCORES = 8
BPC = B // NCORES          # samples per core = 4
PAIRS = BPC // 2           # sample pairs per core = 2
CIN = COUT = 64
H = W = 128
HP = WP = 130              # padded dims
KH = KW = 3
NT = KH * KW               # taps = 9
CL = 8                     # c length
CL1 = CL + 1               # + ones row
MH = 16                    # mlp hidden
K2 = MH + 1                # mlp hidden + ones row
NCH = (H * W) // 512       # 512-col output chunks per pair = 32
W2C = NT * CIN * COUT      # 36864 columns of w2

_CACHE = {}


def _build():
    import concourse.bass as bass
    import concourse.mybir as mybir
    import concourse.tile as tile
    from concourse import bacc
    from concourse.tile_rust import add_dep_helper

    f32 = mybir.dt.float32
    f32r = mybir.dt.float32r
    bf16 = mybir.dt.bfloat16
    AF = mybir.ActivationFunctionType

    nc = bacc.Bacc("TRN2", target_bir_lowering=False, debug=False)

    xs_d = nc.dram_tensor("xsp", [BPC, CIN, HP * WP], bf16, kind="ExternalInput")
    ct_d = nc.dram_tensor("cT", [CL1, 32], f32, kind="ExternalInput")
    w1_d = nc.dram_tensor("w1", [CL1, 128], f32, kind="ExternalInput")
    b1_d = nc.dram_tensor("b1", [128, 1], f32, kind="ExternalInput")
    w2_d = nc.dram_tensor("w2p", [68, W2C // 4], bf16, kind="ExternalInput")
    zz_d = nc.dram_tensor("zz", [1, NT * 128], bf16, kind="ExternalInput")
    cb_d = nc.dram_tensor("cb2", [128, 1], f32, kind="ExternalInput")
    out_d = nc.dram_tensor("out", [BPC, COUT, H, W], bf16, kind="ExternalOutput")

    with tile.TileContext(nc) as tc:
        with (
            tc.tile_pool(name="consts", bufs=1) as consts,
            tc.tile_pool(name="adjpool", bufs=2) as adjpool,
            tc.tile_pool(name="xpool", bufs=1) as xpool,
            tc.tile_pool(name="opool", bufs=10) as opool,
            tc.tile_pool(name="pspool", bufs=1, space=bass.MemorySpace.PSUM) as ps,
        ):
            # ---- act-table + PE p-state warmups (off the critical path) ----
            warm = consts.tile([1, 1], f32)
            nc.vector.memset(warm[:], 0.0)
            nc.scalar.activation(out=warm[:], in_=warm[:], func=AF.Relu)
            wdum = consts.tile([1, 512], bf16)
            nc.vector.memset(wdum[:], 0.0)
            for _ in range(4):
                pd = ps.tile([BPC, 512], f32, tag="ps", bufs=8)
                nc.tensor.matmul(
                    pd[0:1, :], wdum[0:1, 0:1], wdum[:], start=True, stop=True
                )

            # ---- constants in (sync queue; small + early) ----
            ct_sb = consts.tile([CL1, 32], f32)
            nc.sync.dma_start(out=ct_sb[:], in_=ct_d.ap())
            w1_sb = consts.tile([CL1, 128], f32)
            nc.sync.dma_start(out=w1_sb[:], in_=w1_d.ap())
            b1_sb = consts.tile([128, 1], f32)
            nc.sync.dma_start(out=b1_sb[:], in_=b1_d.ap())
            cb_sb = consts.tile([128, 1], f32)
            cb_dma = nc.sync.dma_start(out=cb_sb[:], in_=cb_d.ap())

            # ---- w2 (k-group packed, only the 17 real rows per group
            # shipped) on the ACT queue; the tap-0-critical 139KB piece
            # first, gated behind the consts so its bulk doesn't delay the
            # tiny ct/w1 completions that gate the MLP ----
            w2s = consts.tile([128, W2C // 4], bf16)
            w2chunks = [
                (0, 0, 4096),       # group 0, tap0-critical columns
                (0, 4096, W2C // 4),
                (1, 0, W2C // 4),
                (2, 0, W2C // 4),
                (3, 0, W2C // 4),
            ]
            w2first = None
            for g, c0, c1 in w2chunks:
                w2dma = nc.scalar.dma_start(
                    out=w2s[32 * g : 32 * g + K2, c0:c1],
                    in_=w2_d.ap()[17 * g : 17 * g + K2, c0:c1],
                )
                add_dep_helper(
                    w2dma.ins, cb_dma.ins, sync=True, reason="consts first"
                )
                if w2first is None:
                    w2first = w2dma

            # per-pair block-diag bf16 weights; off-diag zero-filled via
            # broadcast DMA (SWDGE), also gated behind the consts
            wblk = []
            for p in range(PAIRS):
                wb = consts.tile([128, NT * 128], bf16, name=f"wblk{p}", tag=f"wblk{p}")
                zsrc = bass.AP(
                    tensor=zz_d.ap().tensor, offset=0, ap=[[0, 128], [1, NT * 128]]
                )
                zf = nc.gpsimd.dma_start(out=wb[:], in_=zsrc)
                add_dep_helper(zf.ins, cb_dma.ins, sync=True, reason="consts first")
                wblk.append(wb)

            # ---- bulk x loads: 13-row chunks on sync HWDGE, paced by
            # emission order (first two immediately, the rest interleaved
            # between stage-A taps) so they never congest the small
            # latency-critical loads that gate the MLP ----
            xps = []
            for p in range(PAIRS):
                xp = xpool.tile([128, HP * WP], bf16, name=f"xp{p}")
                xps.append(xp)

            def load_x_chunk(eng, p, k, after=None):
                inst = eng.dma_start(
                    out=xps[p][:, k * 1690 : (k + 1) * 1690],
                    in_=xs_d.ap()[2 * p : 2 * p + 2].rearrange(
                        "b c (k e) -> b c k e", e=1690
                    )[:, :, k, :],
                )
                if after is not None:
                    add_dep_helper(
                        inst.ins, after.ins, sync=True, reason="pace bulk x"
                    )
                return inst

            # first two chunks gated on the tap-0 w2 piece completing so
            # their bulk doesn't congest the loads that gate stage A
            load_x_chunk(nc.sync, 0, 0, after=w2first)
            load_x_chunk(nc.sync, 0, 1, after=w2first)

            # ---- stage A: conditioning MLP (f32r) ----
            # w1 is host-replicated 4x along its free dim, so ph/ht land
            # pre-replicated at partition offsets 0/32/64/96 for the packed
            # w2 k-groups — no replication DMAs needed.
            # ct is host-padded to 32 cols so ph/ht are [128, 32] and each
            # adj matmul can emit a fully-written 32-partition PSUM block
            # (M=32); the junk in pad cols lands on partitions never read
            # by the scatters.
            ph = ps.tile([128, 32], f32, tag="ps", bufs=8)
            nc.tensor.matmul(ph[:], w1_sb[:], ct_sb[:], start=True, stop=True)
            ht_sb = consts.tile([128, 32], bf16)
            nc.scalar.activation(
                out=ht_sb[:], in_=ph[:], func=AF.Relu, bias=b1_sb[:]
            )

            # adj[b, t, ci, co] = sum_k hT[k, b] w2p[k, t, ci, co]
            # (w2p row 16 carries mlp_b2 + conv_w, so adj is the full
            # weight). The matmul PSUM dst must start at partition 0, so
            # adj lands on 4 partitions; casts are element-bound — give
            # the PE 4 pa slots so it never waits on them, and split the
            # casts across DVE / scalar-ACT.
            scat_last = [{}, {}]
            adj_last = {}
            for t in range(NT):
                adj = adjpool.tile([BPC, CIN * COUT], bf16)
                for m in range(8):
                    pa = ps.tile([BPC, 512], f32, tag="ps", bufs=8)
                    j = t * CIN * COUT + m * 512
                    g, col = divmod(j, W2C // 4)
                    adj_last[t] = nc.tensor.matmul(
                        pa[:],
                        ht_sb[32 * g : 32 * g + K2, 0:BPC],
                        w2s[32 * g : 32 * g + K2, col : col + 512],
                        start=True,
                        stop=True,
                        tile_position=(32 * g, 0),
                    )
                    if m % 2 == 0:
                        nc.vector.tensor_copy(
                            adj[:, m * 512 : (m + 1) * 512], pa[:]
                        )
                    else:
                        nc.scalar.activation(
                            out=adj[:, m * 512 : (m + 1) * 512],
                            in_=pa[:],
                            func=AF.Identity,
                        )
                # scatter each sample's [ci, co] block onto wblk's diagonal:
                # pair-0 (latency-critical) on sync HWDGE, pair-1 (needed
                # ~100us later) on the gpsimd SWDGE
                scat = None
                for b in range(BPC):
                    p, half = divmod(b, 2)
                    q = half * 64
                    inst = (nc.sync if p == 0 else nc.gpsimd).dma_start(
                        out=wblk[p][q : q + 64, t * 128 + q : t * 128 + q + 64],
                        in_=adj[b : b + 1, :],
                    )
                    scat_last[p][t] = inst
                    if p == 0:
                        scat = inst
                # pace the remaining pair-0 x chunks behind the scatters so
                # the static schedule can't front-load the bulk x traffic
                # (sync queue: the scalar ENGINE must stay pure-cast during
                # stage A — a DMA issue stall there throttles the tap rate)
                if t < 8:
                    load_x_chunk(nc.sync, 0, t + 2, after=scat)

            # ---- stage B: per-pair conv ----
            last_store = None
            for p in range(PAIRS):
                xp3 = xps[p].rearrange("p (h w) -> p h w", w=WP)
                for g in range(NCH // 4):
                    # pair-1 x chunks trickle in on sync during pair-0 conv,
                    # paced behind the rolling output stores
                    if p + 1 < PAIRS and g >= 1 and g <= 5:
                        load_x_chunk(nc.sync, p + 1, 2 * (g - 1), after=last_store)
                        load_x_chunk(nc.sync, p + 1, 2 * (g - 1) + 1, after=last_store)
                    pos = [
                        ps.tile([128, 512], f32, tag="ps", bufs=8, name=f"po{p}_{g}_{j}")
                        for j in range(4)
                    ]
                    for t in range(NT):
                        kh, kw = divmod(t, 3)
                        for j in range(4):
                            h0 = (g * 4 + j) * 4
                            mm = nc.tensor.matmul(
                                pos[j][:],
                                wblk[p][:, t * 128 : (t + 1) * 128],
                                xp3[:, h0 + kh : h0 + kh + 4, kw : kw + W],
                                start=(t == 0),
                                stop=(t == NT - 1),
                            )
                            # barrier: conv starts only after stage A's
                            # last adj matmul — interleaving the small-tile
                            # adj matmuls with conv drags every matmul to a
                            # just-in-time semaphore chain (~460ns each);
                            # a clean split streams conv at full rate
                            if t == 0:
                                add_dep_helper(
                                    mm.ins, adj_last[NT - 1].ins,
                                    sync=True, reason="stageA barrier",
                                )
                    tail = p == PAIRS - 1 and g >= NCH // 4 - 2
                    for j in range(4):
                        h0 = (g * 4 + j) * 4
                        os = opool.tile([128, 512], bf16, name=f"os{p}_{g}_{j}", tag="os")
                        # bias-adds alternate DVE / scalar-ACT so a group's
                        # 4 PSUM slots release promptly (no serialization
                        # behind a single engine at group boundaries)
                        if j % 2 == 1:
                            nc.scalar.activation(
                                out=os[:], in_=pos[j][:], func=AF.Identity,
                                bias=cb_sb[:],
                            )
                        else:
                            nc.vector.tensor_scalar_add(os[:], pos[j][:], cb_sb[:])
                        qeng = nc.sync if (tail and j % 2 == 0) else nc.scalar
                        last_store = qeng.dma_start(
                            out=out_d.ap()[2 * p : 2 * p + 2, :, h0 : h0 + 4, :],
                            in_=os[:],
                        )

    nc.compile()
    return nc


def _get_nc():
    if "nc" not in _CACHE:
        _CACHE["nc"] = _build()
    return _CACHE["nc"]


def _prep(x, c, conv_w, conv_b, mlp_w1, mlp_b1, mlp_w2, mlp_b2):
    import ml_dtypes

    bf16 = ml_dtypes.bfloat16

    x = np.ascontiguousarray(x, dtype=np.float32)
    c = np.ascontiguousarray(c, dtype=np.float32)
    conv_w = np.asarray(conv_w, dtype=np.float32)
    conv_b = np.asarray(conv_b, dtype=np.float32)
    mlp_w1 = np.asarray(mlp_w1, dtype=np.float32)
    mlp_b1 = np.asarray(mlp_b1, dtype=np.float32)
    mlp_w2 = np.asarray(mlp_w2, dtype=np.float32)
    mlp_b2 = np.asarray(mlp_b2, dtype=np.float32)

    # padded bf16 x, flattened spatial
    xsp = np.zeros((B, CIN, HP, WP), dtype=bf16)
    xsp[:, :, 1 : HP - 1, 1 : WP - 1] = x.astype(bf16)
    xsp = xsp.reshape(B, CIN, HP * WP)

    # w1' [CL1, 128]: 4 replicas of [[w1, 0], [0, 1]] at col offsets
    # 0/32/64/96 so the mlp matmul emits ht pre-replicated per k-group
    w19 = np.zeros((CL1, 128), dtype=np.float32)
    for g in range(4):
        w19[:CL, 32 * g : 32 * g + MH] = mlp_w1
        w19[CL, 32 * g + MH] = 1.0
    b117 = np.zeros((128, 1), dtype=np.float32)
    for g in range(4):
        b117[32 * g : 32 * g + MH, 0] = mlp_b1

    # w2p[k, t*4096 + ci*64 + co] = mlp_w2[k, co*576 + ci*9 + t]
    # row 16 = (mlp_b2 + conv_w), same permutation -> adj == full weight
    w2p = mlp_w2.reshape(MH, COUT, CIN, NT).transpose(0, 3, 2, 1)
    b2p = mlp_b2.reshape(COUT, CIN, NT).transpose(2, 1, 0)
    cwp = conv_w.reshape(COUT, CIN, NT).transpose(2, 1, 0)  # [t, ci, co]
    row16 = (b2p + cwp).reshape(1, -1)
    w2p = np.concatenate([w2p.reshape(MH, -1), row16], axis=0).astype(np.float32)
    # pack 4 column-groups into partition groups of 32 (17 used)
    w2g = w2p.reshape(K2, 4, W2C // 4).transpose(1, 0, 2)
    w2pk = np.zeros((68, W2C // 4), dtype=bf16)
    for g in range(4):
        w2pk[17 * g : 17 * g + K2] = w2g[g]
    w2p = w2pk

    zz = np.zeros((1, NT * 128), dtype=bf16)
    cb2 = np.ascontiguousarray(
        np.tile(conv_b.reshape(COUT, 1), (2, 1)), dtype=np.float32
    )

    in_maps = []
    for i in range(NCORES):
        sl = slice(i * BPC, (i + 1) * BPC)
        ct9 = np.zeros((CL1, 32), dtype=np.float32)
        ct9[:CL, :BPC] = c[sl].T
        ct9[CL, :BPC] = 1.0
        in_maps.append(
            {
                "xsp": np.ascontiguousarray(xsp[sl]),
                "cT": np.ascontiguousarray(ct9),
                "w1": w19,
                "b1": b117,
                "w2p": w2p,
                "zz": zz,
                "cb2": cb2,
            }
        )
    return in_maps


def _run(inputs, trace=False):
    from concourse.bass_utils import run_bass_kernel_spmd

    nc = _get_nc()
    in_maps = _prep(**inputs)
    res = run_bass_kernel_spmd(
        nc, in_maps, core_ids=list(range(NCORES)), trace=trace
    )
    out = np.concatenate(
        [np.asarray(res.results[i]["out"]).astype(np.float32) for i in range(NCORES)],
        axis=0,
    )
    return out, res


def kernel(**inputs):
    out, _ = _run(inputs, trace=False)
    return out
